# revision 49
# baseline (speedup 1.0000x reference)
"""Trainium2 Bass kernel for nn_BoxLoss (elementwise CIoU loss).

Contract: kernel(**inputs) takes the FULL unsharded inputs
(predicts_bbox [64,33600,4] f32, targets_bbox [64,33600,4] f32,
valid_masks [64,33600] bool, box_norm [64,33600] f32, cls_norm scalar f32)
and returns (loss_iou scalar f32, iou [64,33600] f32) exactly like the
reference.

Strategy: data-parallel shard over the batch dim across 8 NeuronCores
(8 batches/core).  Each core runs an identical Bass/Tile program over a
flat [128, 2100] layout (268800 elements).  Host deinterleaves the xyxy
boxes into coordinate planes so all DMA is contiguous, and does the final
tiny gather: iou planes are concatenated, the per-partition partial sums
of iou*box_norm are summed and combined with sum(box_norm*mask) to form
the scalar loss.

Engine split per core: ~46 VectorE plane ops (6 custom fused DVE ops are
registered below; all 5 divisions use reciprocal_approx_fast, ~51 ULP),
1 ScalarE op (Arctan), and a scalar_tensor_tensor accum for the loss
partial sums.  Measured on HW via an on-device For_i loop and differencing
two loop counts: ~124 us/exec vs ~33 us DMA roofline (VectorE-bound).
Error vs the f32 reference: iou scale-rel ~3.3e-6, loss rel ~5e-7.
Arctan's hardware range limit of [-pi/2, pi/2] is handled by the
difference identity atan(qb)-atan(qa) = atan(z), z=(wb*ha-wa*hb)/(ha*hb+wa*wb),
plus reciprocal range reduction atan(|z|) = pi/2 - atan(1/|z|) for |z|>1
(the sign of the difference is irrelevant since only its square is used).
"""

import os
import sys

for _p in ("/opt/trn_rl_repo",):
    if _p not in sys.path and os.path.isdir(_p):
        sys.path.insert(0, _p)

import numpy as np

import concourse.bacc as bacc
import concourse.bass as bass
import concourse.mybir as mybir
from concourse.tile import TileContext

B, A = 64, 33600
NCORES = 8
PER_B = B // NCORES          # batches per core
ELEMS = PER_B * A            # 268800 elements per core
P = 128                      # SBUF partitions
N = ELEMS // P               # 2100 free-dim elements per core
F = int(os.environ.get("BOX_F", "1050"))   # tile free size
T = N // F                                 # tiles per core
WORK_BUFS = int(os.environ.get("BOX_WORK_BUFS", "2"))
ACT_RECIP = int(os.environ.get("BOX_ACT_RECIP", "0"))
POOL_OPS = int(os.environ.get("BOX_POOL_OPS", "0"))  # route some TTs to GpSimd
# x|y-packed geometry measured SLOWER on HW (130.9us vs 123.7us) despite a
# better cost-model prediction; default off.
PACK_XY = int(os.environ.get("BOX_PACK", "0"))
# Offload pure-linear plane combinations (union/den/num/final combine) to the
# TensorEngine as +-identity fp32 matmuls accumulating in PSUM.
PE_OFF = int(os.environ.get("BOX_PE", "0"))
PE_PP = int(os.environ.get("BOX_PE_PP", "0"))  # also offload the final combine
PE_R = int(os.environ.get("BOX_PE_R", "1"))    # use float32r (4x faster, ~2^-16)
PE_CHUNK = 350  # matmul N per PSUM bank (<=512)
_INP_DEFAULT = "1" if (PACK_XY and F >= 1050) else ("2" if T > 1 else "1")
INP_BUFS = int(os.environ.get("BOX_INP_BUFS", _INP_DEFAULT))
EPS = 1e-9
PI = float(np.pi)

AF = mybir.ActivationFunctionType
OP = mybir.AluOpType

IN_NAMES = ["x1a", "y1a", "x2a", "y2a", "x1b", "y1b", "x2b", "y2b", "m", "bn"]
# DMA issue order: planes needed by the first compute ops land first
LOAD_ORDER = ["x1a", "x2a", "x1b", "x2b", "y1a", "y2a", "y1b", "y2b", "m", "bn"]
ILV = int(os.environ.get("BOX_ILV", "1"))  # interleave tiles at the arctan hop
# Halving tile-0 first loads starts DVE 1.5us earlier but the start region
# is DMA-bandwidth-paced (360GB/s/core) so total is unchanged; default off.
SPLIT0 = int(os.environ.get("BOX_SPLIT0", "0"))


# --------------------------------------------------------------------------
# Custom fused DVE ops (registered into concourse.dve_ops at import time).
# Each is one VectorE instruction; bodies stay within the 8-ALU-stage budget.
# --------------------------------------------------------------------------

_BOX_OPS = {}


def _register_custom_ops():
    if _BOX_OPS:
        return _BOX_OPS
    import concourse.dve_ops as dve_ops
    from concourse.dve_ops import OPS, CUSTOM_DVE_SPECS, DveOp, _SUB_OPCODE_FOR_NAME, _CUSTOM_DVE_ROW_BASE
    from concourse.dve_spec import (
        Spec, Src0, Src1, C0, C1, Zero, One, relu, sq, maxx, select, lower,
        _has_src1,
    )
    from concourse.dve_uop import DveOpSpec

    def mk(name, spec):
        if name in _SUB_OPCODE_FOR_NAME:
            return next(o for o in OPS if o.name == name)
        opcode = _CUSTOM_DVE_ROW_BASE + len(OPS)
        assert opcode < 0x20
        shas = {}
        for ver in ("v3", "v4"):
            try:
                uops = lower(spec, ver=ver)
                shas[ver] = DveOpSpec(
                    name=name, opcode=opcode, uops=uops, rd1_en=_has_src1(spec)
                ).sha(ver)
            except Exception:
                pass
        op = DveOp(name, spec, subdim=False, uops_sha=shas)
        OPS.append(op)
        _SUB_OPCODE_FOR_NAME[name] = opcode
        CUSTOM_DVE_SPECS[name] = spec
        return op

    import numpy as _np

    # inter = relu(iwr) * relu(ihr)
    _BOX_OPS["RELUMUL"] = mk("BOX_RELUMUL", Spec(
        body=relu(Src0) * relu(Src1),
        reference=lambda in0, in1, c0, c1, c2:
            (_np.maximum(in0, 0) * _np.maximum(in1, 0)).astype(_np.float32),
    ))
    # c2 / rho2: (in0*c0)^2 + (in1*c0)^2
    _BOX_OPS["SQADD"] = mk("BOX_SQADD", Spec(
        body=sq(Src0 * C0) + sq(Src1 * C0),
        reference=lambda in0, in1, c0, c1, c2:
            ((in0 * c0) ** 2 + (in1 * c0) ** 2).astype(_np.float32),
    ))
    # azc = max(|num * r_den|, c0)
    _m = Src0 * Src1
    _BOX_OPS["ZABSC"] = mk("BOX_ZABSC", Spec(
        body=maxx(maxx(_m, Zero - _m), C0),
        reference=lambda in0, in1, c0, c1, c2:
            _np.maximum(_np.abs(in0 * in1), c0).astype(_np.float32),
    ))
    # v = (select(azc > 1, pi/2 - t, t) * c1)^2
    _BOX_OPS["VQ"] = mk("BOX_VQ", Spec(
        body=sq(select(Src1 > One, C0 - Src0, Src0) * C1),
        reference=lambda in0, in1, c0, c1, c2:
            ((_np.where(in1 > 1.0, c0 - in0, in0) * c1) ** 2).astype(_np.float32),
    ))
    # denom = ((v - iou) + 1) + c0
    _BOX_OPS["DENOM"] = mk("BOX_DENOM", Spec(
        body=((Src0 - Src1) + One) + C0,
        reference=lambda in0, in1, c0, c1, c2:
            (((in0 - in1) + _np.float32(1.0)) + c0).astype(_np.float32),
    ))
    # alphav = v^2 * r_denom
    _BOX_OPS["SQMUL"] = mk("BOX_SQMUL", Spec(
        body=sq(Src0) * Src1,
        reference=lambda in0, in1, c0, c1, c2:
            (in0 * in0 * in1).astype(_np.float32),
    ))
    return _BOX_OPS


_register_custom_ops()


def _act_recip(nc, out, in_):
    """ScalarE Reciprocal via raw InstActivation (the nc.scalar.activation
    builder refuses Reciprocal due to spline accuracy ~hundreds of ULP; that
    is acceptable here and offloads the DVE)."""
    sc = nc.scalar
    ins = [
        sc.lower_ap(in_),
        mybir.ImmediateValue(dtype=mybir.dt.float32, value=0.0),   # bias
        mybir.ImmediateValue(dtype=mybir.dt.float32, value=1.0),   # scale
        mybir.ImmediateValue(dtype=mybir.dt.float32, value=0.0),   # alpha
    ]
    return sc.add_instruction(
        mybir.InstActivation(
            name=nc.get_next_instruction_name(),
            func=AF.Reciprocal,
            ins=ins,
            outs=[sc.lower_ap(out)],
        )
    )


class _Slots:
    """Tiny register allocator over a shared tile pool: a fixed set of
    recycled tags so SBUF usage stays at peak-liveness, not total-plane
    count."""

    def __init__(self, pool, dtype):
        self.pool = pool
        self.dtype = dtype
        self.free = {}
        self.next_id = 0
        self.live = {}

    def alloc(self, name, w=None):
        w = F if w is None else w
        key = "w" if w == F else f"w{w}"
        lst = self.free.setdefault(key, [])
        if lst:
            tag = lst.pop()
        else:
            tag = f"slot_{key}_{self.next_id}"
            self.next_id += 1
        t = self.pool.tile([P, w], self.dtype, tag=tag)
        self.live[name] = (t, tag, key)
        return t

    def __getitem__(self, name):
        return self.live[name][0]

    def drop(self, *names):
        for name in names:
            t, tag, key = self.live.pop(name)
            self.free.setdefault(key, []).append(tag)


def _emit_tile(nc, s, ins, iou_out_dram, acc_prev, acc_out, ti, pe=None, split=False):
    """Emit one [P, F] tile worth of the CIoU program.

    ins: dict name -> SBUF tile ([P, F]) for the 10 inputs.
    acc_prev: None or [P,1] AP with the running loss partial sums.
    acc_out: [P,1] AP to write the updated partial sums.
    pe: None, or (psum_pool, wpos_tile, wneg_tile) to offload linear
    combines to the TensorEngine.
    """
    v = nc.vector
    sc = nc.scalar
    NCH = F // PE_CHUNK

    def pe_combine(name, terms):
        """PSUM chunks of sum(sign*plane) via +-identity fp32 matmuls."""
        psum_pool, wpos, wneg = pe
        chunks = []
        for c in range(NCH):
            pt = psum_pool.tile([P, PE_CHUNK], mybir.dt.float32, tag=f"{name}{c}")
            sl = slice(c * PE_CHUNK, (c + 1) * PE_CHUNK)
            for i, (src, sign) in enumerate(terms):
                w = wneg if sign < 0 else wpos
                w_ap, s_ap = w[:], src[:][:, sl]
                if PE_R:
                    w_ap = w_ap.bitcast(mybir.dt.float32r)
                    s_ap = s_ap.bitcast(mybir.dt.float32r)
                nc.tensor.matmul(
                    pt[:], w_ap, s_ap,
                    start=(i == 0), stop=(i == len(terms) - 1),
                )
            chunks.append(pt)
        return chunks

    def tt(dst, a, b, op):
        t = s.alloc(dst)
        v.tensor_tensor(t[:], a[:], b[:], op)
        return t

    def ttp(dst, a, b, op):
        t = s.alloc(dst)
        eng = nc.gpsimd if POOL_OPS else nc.vector
        eng.tensor_tensor(t[:], a[:], b[:], op)
        return t

    # --- geometry x ---
    if SPLIT0 and ti == 0:
        half = F // 2
        wa = s.alloc("wa")
        wb = s.alloc("wb")
        for lo, hi in ((0, half), (half, F)):
            v.tensor_tensor(wa[:][:, lo:hi], ins["x2a"][:][:, lo:hi],
                            ins["x1a"][:][:, lo:hi], OP.subtract)
            v.tensor_tensor(wb[:][:, lo:hi], ins["x2b"][:][:, lo:hi],
                            ins["x1b"][:][:, lo:hi], OP.subtract)
    else:
        wa = tt("wa", ins["x2a"], ins["x1a"], OP.subtract)
        wb = tt("wb", ins["x2b"], ins["x1b"], OP.subtract)
    ix1 = ttp("ix1", ins["x1a"], ins["x1b"], OP.max)
    ix2 = ttp("ix2", ins["x2a"], ins["x2b"], OP.min)
    iwr = tt("iwr", ix2, ix1, OP.subtract)
    s.drop("ix1", "ix2")
    swx = tt("swx", wa, wb, OP.add)
    cw = tt("cw", swx, iwr, OP.subtract)
    s.drop("swx")
    d1x = tt("d1x", ins["x1b"], ins["x1a"], OP.subtract)
    dwx = tt("dwx", wb, wa, OP.subtract)
    dxr = s.alloc("dxr")
    v.affine_then_add(dxr[:], d1x[:], dwx[:], 2.0, 0.0)  # 2*d1x + dwx
    s.drop("d1x", "dwx")

    # --- geometry y ---
    ha = tt("ha", ins["y2a"], ins["y1a"], OP.subtract)
    hb = tt("hb", ins["y2b"], ins["y1b"], OP.subtract)
    iy1 = ttp("iy1", ins["y1a"], ins["y1b"], OP.max)
    iy2 = ttp("iy2", ins["y2a"], ins["y2b"], OP.min)
    ihr = tt("ihr", iy2, iy1, OP.subtract)
    s.drop("iy1", "iy2")
    swy = tt("swy", ha, hb, OP.add)
    ch = tt("ch", swy, ihr, OP.subtract)
    s.drop("swy")
    d1y = tt("d1y", ins["y1b"], ins["y1a"], OP.subtract)
    dwy = tt("dwy", hb, ha, OP.subtract)
    dyr = s.alloc("dyr")
    v.affine_then_add(dyr[:], d1y[:], dwy[:], 2.0, 0.0)
    s.drop("d1y", "dwy")

    # --- intersection / union / iou ---
    inter = s.alloc("inter")
    v._custom_dve(_BOX_OPS["RELUMUL"], out=inter[:], in0=iwr[:], in1=ihr[:])
    s.drop("iwr", "ihr")
    area_a = tt("area_a", wa, ha, OP.mult)
    area_b = tt("area_b", wb, hb, OP.mult)
    r_union = s.alloc("r_union")
    if pe is not None:
        un_ch = pe_combine("un", [(area_a, 1), (area_b, 1), (inter, -1)])
        s.drop("area_a", "area_b")
        for c in range(NCH):
            sl = slice(c * PE_CHUNK, (c + 1) * PE_CHUNK)
            v.reciprocal_approx_fast(r_union[:][:, sl], un_ch[c][:])
    else:
        ab = tt("ab", area_a, area_b, OP.add)
        s.drop("area_a", "area_b")
        union = tt("union", ab, inter, OP.subtract)   # >= 16, +EPS is an f32 no-op
        s.drop("ab")
        if ACT_RECIP:
            _act_recip(nc, r_union[:], union[:])
        else:
            v.reciprocal_approx_fast(r_union[:], union[:])
        s.drop("union")
    iou = tt("iou", inter, r_union, OP.mult)
    s.drop("inter", "r_union")

    # --- enclosing box diagonal + center distance term ---
    c2 = s.alloc("c2")
    v._custom_dve(_BOX_OPS["SQADD"], out=c2[:], in0=cw[:], in1=ch[:], s0=1.0)
    s.drop("cw", "ch")
    r_c2 = s.alloc("r_c2")
    if ACT_RECIP:
        _act_recip(nc, r_c2[:], c2[:])
    else:
        v.reciprocal_approx_fast(r_c2[:], c2[:])
    s.drop("c2")
    rho2 = s.alloc("rho2")
    v._custom_dve(_BOX_OPS["SQADD"], out=rho2[:], in0=dxr[:], in1=dyr[:], s0=0.5)
    s.drop("dxr", "dyr")
    term_rho = tt("term_rho", rho2, r_c2, OP.mult)
    s.drop("rho2", "r_c2")

    # --- aspect-ratio term (arctan via difference identity) ---
    n1 = ttp("n1", wb, ha, OP.mult)
    n2 = ttp("n2", wa, hb, OP.mult)
    dd1 = ttp("dd1", ha, hb, OP.mult)
    dd2 = ttp("dd2", wa, wb, OP.mult)
    azc = s.alloc("azc")
    if pe is not None:
        dn_ch = pe_combine("dn", [(dd1, 1), (dd2, 1)])
        s.drop("dd1", "dd2", "wa", "wb", "ha", "hb")
        r_den = s.alloc("r_den")
        for c in range(NCH):
            sl = slice(c * PE_CHUNK, (c + 1) * PE_CHUNK)
            v.reciprocal_approx_fast(r_den[:][:, sl], dn_ch[c][:])
        nm_ch = pe_combine("un", [(n1, 1), (n2, -1)])
        s.drop("n1", "n2")
        for c in range(NCH):
            sl = slice(c * PE_CHUNK, (c + 1) * PE_CHUNK)
            v._custom_dve(_BOX_OPS["ZABSC"], out=azc[:][:, sl],
                          in0=nm_ch[c][:], in1=r_den[:][:, sl], s0=1e-12)
        s.drop("r_den")
    else:
        num = tt("num", n1, n2, OP.subtract)
        s.drop("n1", "n2")
        den = tt("den", dd1, dd2, OP.add)             # >= 32
        s.drop("dd1", "dd2", "wa", "wb", "ha", "hb")
        r_den = s.alloc("r_den")
        if ACT_RECIP:
            _act_recip(nc, r_den[:], den[:])
        else:
            v.reciprocal_approx_fast(r_den[:], den[:])
        s.drop("den")
        v._custom_dve(_BOX_OPS["ZABSC"], out=azc[:], in0=num[:], in1=r_den[:],
                      s0=1e-12)
        s.drop("num", "r_den")
    rz = s.alloc("rz")
    v.reciprocal_approx_fast(rz[:], azc[:])
    a = tt("a", azc, rz, OP.min)                      # min(|z|, 1/|z|) in [0,1]
    s.drop("rz")
    t_at = s.alloc("t_at")
    sc.activation(t_at[:], a[:], AF.Arctan)
    s.drop("a")
    if split:
        # live across the interleave: t_at, azc, iou, term_rho
        return
    _emit_tile_post(nc, s, ins, iou_out_dram, acc_out, ti, pe)


def _emit_tile_post(nc, s, ins, iou_out_dram, acc_out, ti, pe=None):
    v = nc.vector

    def tt(dst, a, b, op):
        t = s.alloc(dst)
        v.tensor_tensor(t[:], a[:], b[:], op)
        return t

    t_at, azc, iou, term_rho = s["t_at"], s["azc"], s["iou"], s["term_rho"]
    vv = s.alloc("vv")
    # v = (4/pi^2) * (azc>1 ? pi/2 - t : t)^2
    v._custom_dve(_BOX_OPS["VQ"], out=vv[:], in0=t_at[:], in1=azc[:],
                  s0=PI / 2.0, s1=2.0 / PI)
    s.drop("t_at", "azc")

    # --- alpha * v ---
    denom = s.alloc("denom")
    # ((v - iou) + 1) + EPS, association chosen so iou==1, v==0 -> EPS
    v._custom_dve(_BOX_OPS["DENOM"], out=denom[:], in0=vv[:], in1=iou[:], s0=EPS)
    r_denom = s.alloc("r_denom")
    v.reciprocal_approx_fast(r_denom[:], denom[:])
    s.drop("denom")
    alphav = s.alloc("alphav")
    v._custom_dve(_BOX_OPS["SQMUL"], out=alphav[:], in0=vv[:], in1=r_denom[:])
    s.drop("vv", "r_denom")

    # --- combine, mask, output, loss partial ---
    if pe is not None and PE_PP:
        pp_ch = pe_combine("dn", [(iou, 1), (term_rho, -1), (alphav, -1)])
        s.drop("iou", "term_rho", "alphav")
        iou_out = s.alloc("iou_out")
        for c in range(NCH):
            sl = slice(c * PE_CHUNK, (c + 1) * PE_CHUNK)
            v.tensor_tensor(iou_out[:][:, sl], pp_ch[c][:],
                            ins["m"][:][:, sl], OP.mult)
    else:
        p1 = tt("p1", iou, term_rho, OP.subtract)
        s.drop("iou", "term_rho")
        p2 = tt("p2", p1, alphav, OP.subtract)
        s.drop("p1", "alphav")
        iou_out = tt("iou_out", p2, ins["m"], OP.mult)
        s.drop("p2")
    nc.sync.dma_start(iou_out_dram[:, ti * F:(ti + 1) * F], iou_out[:])

    scratch = s.alloc("scratch")
    # lp = iou_out * bn, accum_out = per-partition sum(lp)
    v.scalar_tensor_tensor(
        scratch[:], iou_out[:], 0.0, ins["bn"][:], OP.add, OP.mult,
        accum_out=acc_out,
    )
    s.drop("scratch", "iou_out")


class _V:
    """View wrapper so `x[:]` returns a fixed AP (used for packed halves)."""

    def __init__(self, ap):
        self._ap = ap

    def __getitem__(self, _):
        return self._ap


def _emit_tile_packed(nc, s, ins, iou_out_dram, acc_prev, acc_out, ti):
    """Like _emit_tile but the x/y geometry runs on [P, 2F] packed tiles
    ([x-half | y-half]), halving the geometry instruction count."""
    v = nc.vector
    sc = nc.scalar
    F2 = 2 * F

    def tt2(dst, a, b, op):
        t = s.alloc(dst, w=F2)
        v.tensor_tensor(t[:], a[:], b[:], op)
        return t

    def tt(dst, a, b, op):
        t = s.alloc(dst)
        v.tensor_tensor(t[:], a[:], b[:], op)
        return t

    p1a, p2a, p1b, p2b = ins["p1a"], ins["p2a"], ins["p1b"], ins["p2b"]
    # --- geometry (x|y packed), eagerly consumed to cap SBUF liveness ---
    WHA = tt2("WHA", p2a, p1a, OP.subtract)           # [wa | ha]
    WHB = tt2("WHB", p2b, p1b, OP.subtract)           # [wb | hb]
    wa, ha = _V(WHA[:, 0:F]), _V(WHA[:, F:F2])
    wb, hb = _V(WHB[:, 0:F]), _V(WHB[:, F:F2])

    I1 = tt2("I1", p1a, p1b, OP.max)                  # [ix1 | iy1]
    I2 = tt2("I2", p2a, p2b, OP.min)                  # [ix2 | iy2]
    IR = tt2("IR", I2, I1, OP.subtract)               # [iwr | ihr]
    s.drop("I1", "I2")
    iwr, ihr = _V(IR[:, 0:F]), _V(IR[:, F:F2])
    SW = tt2("SW", WHA, WHB, OP.add)                  # [swx | swy]
    CWH = tt2("CWH", SW, IR, OP.subtract)             # [cw | ch]
    s.drop("SW")
    cw, ch = _V(CWH[:, 0:F]), _V(CWH[:, F:F2])
    inter = s.alloc("inter")
    v._custom_dve(_BOX_OPS["RELUMUL"], out=inter[:], in0=iwr[:], in1=ihr[:])
    s.drop("IR")
    c2 = s.alloc("c2")
    v._custom_dve(_BOX_OPS["SQADD"], out=c2[:], in0=cw[:], in1=ch[:], s0=1.0)
    s.drop("CWH")

    D1 = tt2("D1", p1b, p1a, OP.subtract)             # [x1b-x1a | y1b-y1a]
    DW = tt2("DW", WHB, WHA, OP.subtract)             # [dwx | dwy]
    DR = s.alloc("DR", w=F2)
    v.affine_then_add(DR[:], D1[:], DW[:], 2.0, 0.0)  # [dxr | dyr]
    s.drop("D1", "DW")
    dxr, dyr = _V(DR[:, 0:F]), _V(DR[:, F:F2])
    rho2 = s.alloc("rho2")
    v._custom_dve(_BOX_OPS["SQADD"], out=rho2[:], in0=dxr[:], in1=dyr[:], s0=0.5)
    s.drop("DR")

    # --- union / iou ---
    area_a = tt("area_a", wa, ha, OP.mult)
    area_b = tt("area_b", wb, hb, OP.mult)
    ab = tt("ab", area_a, area_b, OP.add)
    s.drop("area_a", "area_b")
    union = tt("union", ab, inter, OP.subtract)
    s.drop("ab")
    r_union = s.alloc("r_union")
    v.reciprocal_approx_fast(r_union[:], union[:])
    s.drop("union")
    iou = tt("iou", inter, r_union, OP.mult)
    s.drop("inter", "r_union")

    # --- center distance term ---
    r_c2 = s.alloc("r_c2")
    v.reciprocal_approx_fast(r_c2[:], c2[:])
    s.drop("c2")
    term_rho = tt("term_rho", rho2, r_c2, OP.mult)
    s.drop("rho2", "r_c2")

    # --- aspect-ratio term ---
    n1 = tt("n1", wb, ha, OP.mult)
    n2 = tt("n2", wa, hb, OP.mult)
    num = tt("num", n1, n2, OP.subtract)
    s.drop("n1", "n2")
    dd1 = tt("dd1", ha, hb, OP.mult)
    dd2 = tt("dd2", wa, wb, OP.mult)
    den = tt("den", dd1, dd2, OP.add)
    s.drop("dd1", "dd2", "WHA", "WHB")
    r_den = s.alloc("r_den")
    v.reciprocal_approx_fast(r_den[:], den[:])
    s.drop("den")
    azc = s.alloc("azc")
    v._custom_dve(_BOX_OPS["ZABSC"], out=azc[:], in0=num[:], in1=r_den[:], s0=1e-12)
    s.drop("num", "r_den")
    rz = s.alloc("rz")
    v.reciprocal_approx_fast(rz[:], azc[:])
    a = tt("a", azc, rz, OP.min)
    s.drop("rz")
    t_at = s.alloc("t_at")
    sc.activation(t_at[:], a[:], AF.Arctan)
    s.drop("a")
    vv = s.alloc("vv")
    v._custom_dve(_BOX_OPS["VQ"], out=vv[:], in0=t_at[:], in1=azc[:],
                  s0=PI / 2.0, s1=2.0 / PI)
    s.drop("t_at", "azc")

    # --- alpha * v ---
    denom = s.alloc("denom")
    v._custom_dve(_BOX_OPS["DENOM"], out=denom[:], in0=vv[:], in1=iou[:], s0=EPS)
    r_denom = s.alloc("r_denom")
    v.reciprocal_approx_fast(r_denom[:], denom[:])
    s.drop("denom")
    alphav = s.alloc("alphav")
    v._custom_dve(_BOX_OPS["SQMUL"], out=alphav[:], in0=vv[:], in1=r_denom[:])
    s.drop("vv", "r_denom")

    # --- combine, mask, output, loss partial ---
    p1 = tt("p1", iou, term_rho, OP.subtract)
    s.drop("iou", "term_rho")
    p2 = tt("p2", p1, alphav, OP.subtract)
    s.drop("p1", "alphav")
    iou_out = tt("iou_out", p2, ins["m"], OP.mult)
    s.drop("p2")
    nc.sync.dma_start(iou_out_dram[:, ti * F:(ti + 1) * F], iou_out[:])

    scratch = s.alloc("scratch")
    v.scalar_tensor_tensor(
        scratch[:], iou_out[:], 0.0, ins["bn"][:], OP.add, OP.mult,
        accum_out=acc_out,
    )
    s.drop("scratch", "iou_out")


def build_bass(loop_n=None):
    """Build the per-core Bass program (identical on all 8 cores).

    loop_n: if set, wrap the whole program in a hardware For_i loop that
    re-runs it loop_n times (for timing measurement only — outputs are
    rewritten with identical values each iteration).
    """
    nc = bacc.Bacc("TRN2", target_bir_lowering=False)
    din = {
        nm: nc.dram_tensor(nm, [P, N], mybir.dt.float32, kind="ExternalInput").ap()
        for nm in IN_NAMES
    }
    iou_out_d = nc.dram_tensor(
        "iou_out", [P, N], mybir.dt.float32, kind="ExternalOutput"
    ).ap()
    acc_d = nc.dram_tensor("acc", [P, 1], mybir.dt.float32, kind="ExternalOutput").ap()
    if PE_OFF:
        wpos_d = nc.dram_tensor("wpos", [P, P], mybir.dt.float32,
                                kind="ExternalInput").ap()
        wneg_d = nc.dram_tensor("wneg", [P, P], mybir.dt.float32,
                                kind="ExternalInput").ap()

    with TileContext(nc) as tc:
        with (
            tc.tile_pool(name="inp", bufs=INP_BUFS) as inp_pool,
            tc.tile_pool(name="work", bufs=WORK_BUFS) as work_pool,
            tc.tile_pool(name="accp", bufs=1) as acc_pool,
            tc.tile_pool(name="ps", bufs=1, space="PSUM") as psum_pool,
            tc.tile_pool(name="wts", bufs=1) as wts_pool,
        ):
            pe = None
            if PE_OFF:
                wpos_t = wts_pool.tile([P, P], mybir.dt.float32, tag="wpos")
                wneg_t = wts_pool.tile([P, P], mybir.dt.float32, tag="wneg")
                nc.sync.dma_start(wpos_t[:], wpos_d[:])
                nc.sync.dma_start(wneg_t[:], wneg_d[:])
                pe = (psum_pool, wpos_t, wneg_t)
            def body(_iv=None):
                s = _Slots(work_pool, mybir.dt.float32)
                s_list = [_Slots(work_pool, mybir.dt.float32) for _ in range(T)]
                ins_list = [None] * T
                acc_tiles = []
                for ti in range(T):
                    sl = slice(ti * F, (ti + 1) * F)
                    ins = {}
                    if PACK_XY:
                        pairs = {
                            "p1a": ("x1a", "y1a"), "p2a": ("x2a", "y2a"),
                            "p1b": ("x1b", "y1b"), "p2b": ("x2b", "y2b"),
                        }
                        for pnm, (xn, yn) in pairs.items():
                            t = inp_pool.tile([P, 2 * F], mybir.dt.float32, tag=pnm)
                            nc.sync.dma_start(t[:, 0:F], din[xn][:, sl])
                            nc.sync.dma_start(t[:, F:2 * F], din[yn][:, sl])
                            ins[pnm] = t
                        for nm in ("m", "bn"):
                            t = inp_pool.tile([P, F], mybir.dt.float32, tag=nm)
                            nc.sync.dma_start(t[:], din[nm][:, sl])
                            ins[nm] = t
                    else:
                        half = F // 2
                        split_set = ("x1a", "x2a", "x1b", "x2b") \
                            if (SPLIT0 and ti == 0) else ()
                        for nm in split_set:  # first halves of first-needed
                            t = inp_pool.tile([P, F], mybir.dt.float32, tag=nm)
                            nc.sync.dma_start(
                                t[:, 0:half], din[nm][:, ti * F:ti * F + half])
                            ins[nm] = t
                        for nm in split_set:  # then their second halves
                            nc.sync.dma_start(
                                ins[nm][:, half:F],
                                din[nm][:, ti * F + half:(ti + 1) * F])
                        for nm in LOAD_ORDER:
                            if nm in split_set:
                                continue
                            t = inp_pool.tile([P, F], mybir.dt.float32, tag=nm)
                            nc.sync.dma_start(t[:], din[nm][:, sl])
                            ins[nm] = t
                    acc_t = acc_pool.tile([P, 1], mybir.dt.float32, tag=f"acc{ti}")
                    if PACK_XY:
                        _emit_tile_packed(nc, s, ins, iou_out_d, None, acc_t[:], ti)
                    elif ILV:
                        # phase 1 only (through arctan); post emitted below
                        ins_list[ti] = ins
                        _emit_tile(nc, s_list[ti], ins, iou_out_d, None,
                                   acc_t[:], ti, pe=pe, split=True)
                    else:
                        _emit_tile(nc, s, ins, iou_out_d, None, acc_t[:], ti, pe=pe)
                    acc_tiles.append(acc_t)
                if ILV and not PACK_XY:
                    for ti in range(T):
                        _emit_tile_post(nc, s_list[ti], ins_list[ti], iou_out_d,
                                        acc_tiles[ti][:], ti, pe)
                acc_total = acc_tiles[0]
                for ti in range(1, T):
                    acc_new = acc_pool.tile([P, 1], mybir.dt.float32, tag=f"accsum{ti}")
                    nc.vector.tensor_tensor(
                        acc_new[:], acc_total[:], acc_tiles[ti][:], OP.add
                    )
                    acc_total = acc_new
                nc.sync.dma_start(acc_d[:], acc_total[:])

            if loop_n is None:
                body()
            else:
                with tc.For_i(0, loop_n, 1):
                    body()
    nc.compile()
    return nc


_CACHE = {}


def _get_nc():
    if "nc" not in _CACHE:
        _CACHE["nc"] = build_bass()
    return _CACHE["nc"]


def shard_inputs(predicts_bbox, targets_bbox, valid_masks, box_norm):
    """Deinterleave xyxy into coordinate planes and shard over batch."""
    pb = np.asarray(predicts_bbox, dtype=np.float32)
    tb = np.asarray(targets_bbox, dtype=np.float32)
    m = np.asarray(valid_masks).astype(np.float32)
    bn = np.asarray(box_norm, dtype=np.float32)

    in_maps = []
    for c in range(NCORES):
        sl = slice(c * PER_B, (c + 1) * PER_B)
        d = {}
        for i, nm in enumerate(["x1a", "y1a", "x2a", "y2a"]):
            d[nm] = np.ascontiguousarray(pb[sl, :, i]).reshape(P, N)
        for i, nm in enumerate(["x1b", "y1b", "x2b", "y2b"]):
            d[nm] = np.ascontiguousarray(tb[sl, :, i]).reshape(P, N)
        d["m"] = np.ascontiguousarray(m[sl]).reshape(P, N)
        d["bn"] = np.ascontiguousarray(bn[sl]).reshape(P, N)
        if PE_OFF:
            d["wpos"] = np.eye(P, dtype=np.float32)
            d["wneg"] = -np.eye(P, dtype=np.float32)
        in_maps.append(d)
    return in_maps


def run_sharded(inputs, trace=False, trace_kwargs=None):
    """Run the SPMD kernel on 8 cores; returns (outputs, BassKernelResults)."""
    from concourse import bass_utils

    in_maps = shard_inputs(
        inputs["predicts_bbox"],
        inputs["targets_bbox"],
        inputs["valid_masks"],
        inputs["box_norm"],
    )
    nc = _get_nc()
    kw = {}
    if trace:
        kw["trace"] = True
        if trace_kwargs:
            kw.update(trace_kwargs)
    res = bass_utils.run_bass_kernel_spmd(
        nc, in_maps, core_ids=list(range(NCORES)), **kw
    )
    return res


def assemble_outputs(inputs, results):
    iou = np.concatenate(
        [r["iou_out"].reshape(PER_B, A) for r in results], axis=0
    ).astype(np.float32)

    m = np.asarray(inputs["valid_masks"]).astype(np.float64)
    bn = np.asarray(inputs["box_norm"], dtype=np.float64)
    s1 = float(np.sum(bn * m))                       # sum(box_norm * mask)
    s2 = float(sum(np.sum(r["acc"], dtype=np.float64) for r in results))
    cls_norm = float(np.asarray(inputs["cls_norm"]))
    loss = np.float32((s1 - s2) / cls_norm)
    return loss, iou


def kernel(**inputs):
    res = run_sharded(inputs)
    return assemble_outputs(inputs, res.results)


# revision 51
# speedup vs baseline: 1.0501x; 1.0501x over previous
"""Trainium2 Bass kernel for nn_BoxLoss (elementwise CIoU loss).

Contract: kernel(**inputs) takes the FULL unsharded inputs
(predicts_bbox [64,33600,4] f32, targets_bbox [64,33600,4] f32,
valid_masks [64,33600] bool, box_norm [64,33600] f32, cls_norm scalar f32)
and returns (loss_iou scalar f32, iou [64,33600] f32) exactly like the
reference.

Strategy: data-parallel shard over the batch dim across 8 NeuronCores
(8 batches/core).  Each core runs an identical Bass/Tile program over a
flat [128, 2100] layout (268800 elements).  Host deinterleaves the xyxy
boxes into coordinate planes so all DMA is contiguous, and does the final
tiny gather: iou planes are concatenated, the per-partition partial sums
of iou*box_norm are summed and combined with sum(box_norm*mask) to form
the scalar loss.

Engine split per core: ~46 VectorE plane ops (6 custom fused DVE ops are
registered below; all 5 divisions use reciprocal_approx_fast, ~51 ULP),
1 ScalarE op (Arctan), and a scalar_tensor_tensor accum for the loss
partial sums.  Measured on HW via an on-device For_i loop and differencing
two loop counts: ~124 us/exec vs ~33 us DMA roofline (VectorE-bound).
Error vs the f32 reference: iou scale-rel ~3.3e-6, loss rel ~5e-7.
Arctan's hardware range limit of [-pi/2, pi/2] is handled by the
difference identity atan(qb)-atan(qa) = atan(z), z=(wb*ha-wa*hb)/(ha*hb+wa*wb),
plus reciprocal range reduction atan(|z|) = pi/2 - atan(1/|z|) for |z|>1
(the sign of the difference is irrelevant since only its square is used).
"""

import os
import sys

for _p in ("/opt/trn_rl_repo",):
    if _p not in sys.path and os.path.isdir(_p):
        sys.path.insert(0, _p)

import numpy as np

import concourse.bacc as bacc
import concourse.bass as bass
import concourse.mybir as mybir
from concourse.tile import TileContext

B, A = 64, 33600
NCORES = 8
PER_B = B // NCORES          # batches per core
ELEMS = PER_B * A            # 268800 elements per core
P = 128                      # SBUF partitions
N = ELEMS // P               # 2100 free-dim elements per core
F = int(os.environ.get("BOX_F", "1050"))   # tile free size
T = N // F                                 # tiles per core
WORK_BUFS = int(os.environ.get("BOX_WORK_BUFS", "2"))
ACT_RECIP = int(os.environ.get("BOX_ACT_RECIP", "0"))
POOL_OPS = int(os.environ.get("BOX_POOL_OPS", "0"))  # route some TTs to GpSimd
# x|y-packed geometry measured SLOWER on HW (130.9us vs 123.7us) despite a
# better cost-model prediction; default off.
PACK_XY = int(os.environ.get("BOX_PACK", "0"))
# Offload pure-linear plane combinations (union/den/num/final combine) to the
# TensorEngine as +-identity fp32 matmuls accumulating in PSUM.
PE_OFF = int(os.environ.get("BOX_PE", "0"))
PE_PP = int(os.environ.get("BOX_PE_PP", "0"))  # also offload the final combine
PE_R = int(os.environ.get("BOX_PE_R", "1"))    # use float32r (4x faster, ~2^-16)
PE_CHUNK = 350  # matmul N per PSUM bank (<=512)
_INP_DEFAULT = "1" if (PACK_XY and F >= 1050) else ("2" if T > 1 else "1")
INP_BUFS = int(os.environ.get("BOX_INP_BUFS", _INP_DEFAULT))
EPS = 1e-9
PI = float(np.pi)

AF = mybir.ActivationFunctionType
OP = mybir.AluOpType

IN_NAMES = ["x1a", "y1a", "x2a", "y2a", "x1b", "y1b", "x2b", "y2b", "m", "bn"]
# DMA issue order: planes needed by the first compute ops land first
LOAD_ORDER = ["x1a", "x2a", "x1b", "x2b", "y1a", "y2a", "y1b", "y2b", "m", "bn"]
ILV = int(os.environ.get("BOX_ILV", "1"))  # interleave tiles at the arctan hop
# Halving tile-0 first loads starts DVE 1.5us earlier but the start region
# is DMA-bandwidth-paced (360GB/s/core) so total is unchanged; default off.
SPLIT0 = int(os.environ.get("BOX_SPLIT0", "0"))
TAILSPLIT = int(os.environ.get("BOX_TAILSPLIT", "0"))  # halve last-tile combine+store


# --------------------------------------------------------------------------
# Custom fused DVE ops (registered into concourse.dve_ops at import time).
# Each is one VectorE instruction; bodies stay within the 8-ALU-stage budget.
# --------------------------------------------------------------------------

_BOX_OPS = {}


def _register_custom_ops():
    if _BOX_OPS:
        return _BOX_OPS
    import concourse.dve_ops as dve_ops
    from concourse.dve_ops import OPS, CUSTOM_DVE_SPECS, DveOp, _SUB_OPCODE_FOR_NAME, _CUSTOM_DVE_ROW_BASE
    from concourse.dve_spec import (
        Spec, Src0, Src1, C0, C1, Zero, One, relu, sq, maxx, select, lower,
        _has_src1,
    )
    from concourse.dve_uop import DveOpSpec

    def mk(name, spec):
        if name in _SUB_OPCODE_FOR_NAME:
            return next(o for o in OPS if o.name == name)
        opcode = _CUSTOM_DVE_ROW_BASE + len(OPS)
        assert opcode < 0x20
        shas = {}
        for ver in ("v3", "v4"):
            try:
                uops = lower(spec, ver=ver)
                shas[ver] = DveOpSpec(
                    name=name, opcode=opcode, uops=uops, rd1_en=_has_src1(spec)
                ).sha(ver)
            except Exception:
                pass
        op = DveOp(name, spec, subdim=False, uops_sha=shas)
        OPS.append(op)
        _SUB_OPCODE_FOR_NAME[name] = opcode
        CUSTOM_DVE_SPECS[name] = spec
        return op

    import numpy as _np

    # inter = relu(iwr) * relu(ihr)
    _BOX_OPS["RELUMUL"] = mk("BOX_RELUMUL", Spec(
        body=relu(Src0) * relu(Src1),
        reference=lambda in0, in1, c0, c1, c2:
            (_np.maximum(in0, 0) * _np.maximum(in1, 0)).astype(_np.float32),
    ))
    # c2 / rho2: (in0*c0)^2 + (in1*c0)^2
    _BOX_OPS["SQADD"] = mk("BOX_SQADD", Spec(
        body=sq(Src0 * C0) + sq(Src1 * C0),
        reference=lambda in0, in1, c0, c1, c2:
            ((in0 * c0) ** 2 + (in1 * c0) ** 2).astype(_np.float32),
    ))
    # azc = max(|num * r_den|, c0)
    _m = Src0 * Src1
    _BOX_OPS["ZABSC"] = mk("BOX_ZABSC", Spec(
        body=maxx(maxx(_m, Zero - _m), C0),
        reference=lambda in0, in1, c0, c1, c2:
            _np.maximum(_np.abs(in0 * in1), c0).astype(_np.float32),
    ))
    # v = (select(azc > 1, pi/2 - t, t) * c1)^2
    _BOX_OPS["VQ"] = mk("BOX_VQ", Spec(
        body=sq(select(Src1 > One, C0 - Src0, Src0) * C1),
        reference=lambda in0, in1, c0, c1, c2:
            ((_np.where(in1 > 1.0, c0 - in0, in0) * c1) ** 2).astype(_np.float32),
    ))
    # denom = ((v - iou) + 1) + c0
    _BOX_OPS["DENOM"] = mk("BOX_DENOM", Spec(
        body=((Src0 - Src1) + One) + C0,
        reference=lambda in0, in1, c0, c1, c2:
            (((in0 - in1) + _np.float32(1.0)) + c0).astype(_np.float32),
    ))
    # alphav = v^2 * r_denom
    _BOX_OPS["SQMUL"] = mk("BOX_SQMUL", Spec(
        body=sq(Src0) * Src1,
        reference=lambda in0, in1, c0, c1, c2:
            (in0 * in0 * in1).astype(_np.float32),
    ))
    return _BOX_OPS


_register_custom_ops()


def _act_recip(nc, out, in_):
    """ScalarE Reciprocal via raw InstActivation (the nc.scalar.activation
    builder refuses Reciprocal due to spline accuracy ~hundreds of ULP; that
    is acceptable here and offloads the DVE)."""
    sc = nc.scalar
    ins = [
        sc.lower_ap(in_),
        mybir.ImmediateValue(dtype=mybir.dt.float32, value=0.0),   # bias
        mybir.ImmediateValue(dtype=mybir.dt.float32, value=1.0),   # scale
        mybir.ImmediateValue(dtype=mybir.dt.float32, value=0.0),   # alpha
    ]
    return sc.add_instruction(
        mybir.InstActivation(
            name=nc.get_next_instruction_name(),
            func=AF.Reciprocal,
            ins=ins,
            outs=[sc.lower_ap(out)],
        )
    )


class _Slots:
    """Tiny register allocator over a shared tile pool: a fixed set of
    recycled tags so SBUF usage stays at peak-liveness, not total-plane
    count."""

    def __init__(self, pool, dtype):
        self.pool = pool
        self.dtype = dtype
        self.free = {}
        self.next_id = 0
        self.live = {}

    def alloc(self, name, w=None):
        w = F if w is None else w
        key = "w" if w == F else f"w{w}"
        lst = self.free.setdefault(key, [])
        if lst:
            tag = lst.pop()
        else:
            tag = f"slot_{key}_{self.next_id}"
            self.next_id += 1
        t = self.pool.tile([P, w], self.dtype, tag=tag)
        self.live[name] = (t, tag, key)
        return t

    def __getitem__(self, name):
        return self.live[name][0]

    def drop(self, *names):
        for name in names:
            t, tag, key = self.live.pop(name)
            self.free.setdefault(key, []).append(tag)


def _emit_tile(nc, s, ins, iou_out_dram, acc_prev, acc_out, ti, pe=None, split=False):
    """Emit one [P, F] tile worth of the CIoU program.

    ins: dict name -> SBUF tile ([P, F]) for the 10 inputs.
    acc_prev: None or [P,1] AP with the running loss partial sums.
    acc_out: [P,1] AP to write the updated partial sums.
    pe: None, or (psum_pool, wpos_tile, wneg_tile) to offload linear
    combines to the TensorEngine.
    """
    v = nc.vector
    sc = nc.scalar
    NCH = F // PE_CHUNK

    def pe_combine(name, terms):
        """PSUM chunks of sum(sign*plane) via +-identity fp32 matmuls."""
        psum_pool, wpos, wneg = pe
        chunks = []
        for c in range(NCH):
            pt = psum_pool.tile([P, PE_CHUNK], mybir.dt.float32, tag=f"{name}{c}")
            sl = slice(c * PE_CHUNK, (c + 1) * PE_CHUNK)
            for i, (src, sign) in enumerate(terms):
                w = wneg if sign < 0 else wpos
                w_ap, s_ap = w[:], src[:][:, sl]
                if PE_R:
                    w_ap = w_ap.bitcast(mybir.dt.float32r)
                    s_ap = s_ap.bitcast(mybir.dt.float32r)
                nc.tensor.matmul(
                    pt[:], w_ap, s_ap,
                    start=(i == 0), stop=(i == len(terms) - 1),
                )
            chunks.append(pt)
        return chunks

    def tt(dst, a, b, op):
        t = s.alloc(dst)
        v.tensor_tensor(t[:], a[:], b[:], op)
        return t

    def ttp(dst, a, b, op):
        t = s.alloc(dst)
        eng = nc.gpsimd if POOL_OPS else nc.vector
        eng.tensor_tensor(t[:], a[:], b[:], op)
        return t

    # --- geometry x ---
    if SPLIT0 and ti == 0:
        half = F // 2
        wa = s.alloc("wa")
        wb = s.alloc("wb")
        for lo, hi in ((0, half), (half, F)):
            v.tensor_tensor(wa[:][:, lo:hi], ins["x2a"][:][:, lo:hi],
                            ins["x1a"][:][:, lo:hi], OP.subtract)
            v.tensor_tensor(wb[:][:, lo:hi], ins["x2b"][:][:, lo:hi],
                            ins["x1b"][:][:, lo:hi], OP.subtract)
    else:
        wa = tt("wa", ins["x2a"], ins["x1a"], OP.subtract)
        wb = tt("wb", ins["x2b"], ins["x1b"], OP.subtract)
    ix1 = ttp("ix1", ins["x1a"], ins["x1b"], OP.max)
    ix2 = ttp("ix2", ins["x2a"], ins["x2b"], OP.min)
    iwr = tt("iwr", ix2, ix1, OP.subtract)
    s.drop("ix1", "ix2")
    swx = tt("swx", wa, wb, OP.add)
    cw = tt("cw", swx, iwr, OP.subtract)
    s.drop("swx")
    d1x = tt("d1x", ins["x1b"], ins["x1a"], OP.subtract)
    dwx = tt("dwx", wb, wa, OP.subtract)
    dxr = s.alloc("dxr")
    v.affine_then_add(dxr[:], d1x[:], dwx[:], 2.0, 0.0)  # 2*d1x + dwx
    s.drop("d1x", "dwx")

    # --- geometry y ---
    ha = tt("ha", ins["y2a"], ins["y1a"], OP.subtract)
    hb = tt("hb", ins["y2b"], ins["y1b"], OP.subtract)
    iy1 = ttp("iy1", ins["y1a"], ins["y1b"], OP.max)
    iy2 = ttp("iy2", ins["y2a"], ins["y2b"], OP.min)
    ihr = tt("ihr", iy2, iy1, OP.subtract)
    s.drop("iy1", "iy2")
    swy = tt("swy", ha, hb, OP.add)
    ch = tt("ch", swy, ihr, OP.subtract)
    s.drop("swy")
    d1y = tt("d1y", ins["y1b"], ins["y1a"], OP.subtract)
    dwy = tt("dwy", hb, ha, OP.subtract)
    dyr = s.alloc("dyr")
    v.affine_then_add(dyr[:], d1y[:], dwy[:], 2.0, 0.0)
    s.drop("d1y", "dwy")

    # --- intersection / union / iou ---
    inter = s.alloc("inter")
    v._custom_dve(_BOX_OPS["RELUMUL"], out=inter[:], in0=iwr[:], in1=ihr[:])
    s.drop("iwr", "ihr")
    area_a = tt("area_a", wa, ha, OP.mult)
    area_b = tt("area_b", wb, hb, OP.mult)
    r_union = s.alloc("r_union")
    if pe is not None:
        un_ch = pe_combine("un", [(area_a, 1), (area_b, 1), (inter, -1)])
        s.drop("area_a", "area_b")
        for c in range(NCH):
            sl = slice(c * PE_CHUNK, (c + 1) * PE_CHUNK)
            v.reciprocal_approx_fast(r_union[:][:, sl], un_ch[c][:])
    else:
        ab = tt("ab", area_a, area_b, OP.add)
        s.drop("area_a", "area_b")
        union = tt("union", ab, inter, OP.subtract)   # >= 16, +EPS is an f32 no-op
        s.drop("ab")
        if ACT_RECIP:
            _act_recip(nc, r_union[:], union[:])
        else:
            v.reciprocal_approx_fast(r_union[:], union[:])
        s.drop("union")
    iou = tt("iou", inter, r_union, OP.mult)
    s.drop("inter", "r_union")

    # --- enclosing box diagonal + center distance term ---
    c2 = s.alloc("c2")
    v._custom_dve(_BOX_OPS["SQADD"], out=c2[:], in0=cw[:], in1=ch[:], s0=1.0)
    s.drop("cw", "ch")
    r_c2 = s.alloc("r_c2")
    if ACT_RECIP:
        _act_recip(nc, r_c2[:], c2[:])
    else:
        v.reciprocal_approx_fast(r_c2[:], c2[:])
    s.drop("c2")
    rho2 = s.alloc("rho2")
    v._custom_dve(_BOX_OPS["SQADD"], out=rho2[:], in0=dxr[:], in1=dyr[:], s0=0.5)
    s.drop("dxr", "dyr")
    term_rho = tt("term_rho", rho2, r_c2, OP.mult)
    s.drop("rho2", "r_c2")

    # --- aspect-ratio term (arctan via difference identity) ---
    n1 = ttp("n1", wb, ha, OP.mult)
    n2 = ttp("n2", wa, hb, OP.mult)
    dd1 = ttp("dd1", ha, hb, OP.mult)
    dd2 = ttp("dd2", wa, wb, OP.mult)
    azc = s.alloc("azc")
    if pe is not None:
        dn_ch = pe_combine("dn", [(dd1, 1), (dd2, 1)])
        s.drop("dd1", "dd2", "wa", "wb", "ha", "hb")
        r_den = s.alloc("r_den")
        for c in range(NCH):
            sl = slice(c * PE_CHUNK, (c + 1) * PE_CHUNK)
            v.reciprocal_approx_fast(r_den[:][:, sl], dn_ch[c][:])
        nm_ch = pe_combine("un", [(n1, 1), (n2, -1)])
        s.drop("n1", "n2")
        for c in range(NCH):
            sl = slice(c * PE_CHUNK, (c + 1) * PE_CHUNK)
            v._custom_dve(_BOX_OPS["ZABSC"], out=azc[:][:, sl],
                          in0=nm_ch[c][:], in1=r_den[:][:, sl], s0=1e-12)
        s.drop("r_den")
    else:
        num = tt("num", n1, n2, OP.subtract)
        s.drop("n1", "n2")
        den = tt("den", dd1, dd2, OP.add)             # >= 32
        s.drop("dd1", "dd2", "wa", "wb", "ha", "hb")
        r_den = s.alloc("r_den")
        if ACT_RECIP:
            _act_recip(nc, r_den[:], den[:])
        else:
            v.reciprocal_approx_fast(r_den[:], den[:])
        s.drop("den")
        v._custom_dve(_BOX_OPS["ZABSC"], out=azc[:], in0=num[:], in1=r_den[:],
                      s0=1e-12)
        s.drop("num", "r_den")
    rz = s.alloc("rz")
    v.reciprocal_approx_fast(rz[:], azc[:])
    a = tt("a", azc, rz, OP.min)                      # min(|z|, 1/|z|) in [0,1]
    s.drop("rz")
    t_at = s.alloc("t_at")
    sc.activation(t_at[:], a[:], AF.Arctan)
    s.drop("a")
    if split:
        # live across the interleave: t_at, azc, iou, term_rho
        return
    _emit_tile_post(nc, s, ins, iou_out_dram, acc_out, ti, pe)


def _emit_tile_post(nc, s, ins, iou_out_dram, acc_out, ti, pe=None):
    v = nc.vector

    def tt(dst, a, b, op):
        t = s.alloc(dst)
        v.tensor_tensor(t[:], a[:], b[:], op)
        return t

    t_at, azc, iou, term_rho = s["t_at"], s["azc"], s["iou"], s["term_rho"]
    vv = s.alloc("vv")
    # v = (4/pi^2) * (azc>1 ? pi/2 - t : t)^2
    v._custom_dve(_BOX_OPS["VQ"], out=vv[:], in0=t_at[:], in1=azc[:],
                  s0=PI / 2.0, s1=2.0 / PI)
    s.drop("t_at", "azc")

    # --- alpha * v ---
    denom = s.alloc("denom")
    # ((v - iou) + 1) + EPS, association chosen so iou==1, v==0 -> EPS
    v._custom_dve(_BOX_OPS["DENOM"], out=denom[:], in0=vv[:], in1=iou[:], s0=EPS)
    r_denom = s.alloc("r_denom")
    v.reciprocal_approx_fast(r_denom[:], denom[:])
    s.drop("denom")
    alphav = s.alloc("alphav")
    v._custom_dve(_BOX_OPS["SQMUL"], out=alphav[:], in0=vv[:], in1=r_denom[:])
    s.drop("vv", "r_denom")

    # --- combine, mask, output, loss partial ---
    if pe is not None and PE_PP:
        pp_ch = pe_combine("dn", [(iou, 1), (term_rho, -1), (alphav, -1)])
        s.drop("iou", "term_rho", "alphav")
        iou_out = s.alloc("iou_out")
        for c in range(NCH):
            sl = slice(c * PE_CHUNK, (c + 1) * PE_CHUNK)
            v.tensor_tensor(iou_out[:][:, sl], pp_ch[c][:],
                            ins["m"][:][:, sl], OP.mult)
    elif TAILSPLIT and ti == T - 1:
        half = F // 2
        p1 = s.alloc("p1")
        p2 = s.alloc("p2")
        iou_out = s.alloc("iou_out")
        for lo, hi in ((0, half), (half, F)):
            v.tensor_tensor(p1[:][:, lo:hi], iou[:][:, lo:hi],
                            term_rho[:][:, lo:hi], OP.subtract)
            v.tensor_tensor(p2[:][:, lo:hi], p1[:][:, lo:hi],
                            alphav[:][:, lo:hi], OP.subtract)
            v.tensor_tensor(iou_out[:][:, lo:hi], p2[:][:, lo:hi],
                            ins["m"][:][:, lo:hi], OP.mult)
            nc.sync.dma_start(
                iou_out_dram[:, ti * F + lo:ti * F + hi],
                iou_out[:][:, lo:hi])
        s.drop("iou", "term_rho", "p1", "alphav", "p2")
    else:
        p1 = tt("p1", iou, term_rho, OP.subtract)
        s.drop("iou", "term_rho")
        p2 = tt("p2", p1, alphav, OP.subtract)
        s.drop("p1", "alphav")
        iou_out = tt("iou_out", p2, ins["m"], OP.mult)
        s.drop("p2")
    if not (TAILSPLIT and ti == T - 1):
        nc.sync.dma_start(iou_out_dram[:, ti * F:(ti + 1) * F], iou_out[:])

    scratch = s.alloc("scratch")
    # lp = iou_out * bn, accum_out = per-partition sum(lp)
    v.scalar_tensor_tensor(
        scratch[:], iou_out[:], 0.0, ins["bn"][:], OP.add, OP.mult,
        accum_out=acc_out,
    )
    s.drop("scratch", "iou_out")


class _V:
    """View wrapper so `x[:]` returns a fixed AP (used for packed halves)."""

    def __init__(self, ap):
        self._ap = ap

    def __getitem__(self, _):
        return self._ap


def _emit_tile_packed(nc, s, ins, iou_out_dram, acc_prev, acc_out, ti):
    """Like _emit_tile but the x/y geometry runs on [P, 2F] packed tiles
    ([x-half | y-half]), halving the geometry instruction count."""
    v = nc.vector
    sc = nc.scalar
    F2 = 2 * F

    def tt2(dst, a, b, op):
        t = s.alloc(dst, w=F2)
        v.tensor_tensor(t[:], a[:], b[:], op)
        return t

    def tt(dst, a, b, op):
        t = s.alloc(dst)
        v.tensor_tensor(t[:], a[:], b[:], op)
        return t

    p1a, p2a, p1b, p2b = ins["p1a"], ins["p2a"], ins["p1b"], ins["p2b"]
    # --- geometry (x|y packed), eagerly consumed to cap SBUF liveness ---
    WHA = tt2("WHA", p2a, p1a, OP.subtract)           # [wa | ha]
    WHB = tt2("WHB", p2b, p1b, OP.subtract)           # [wb | hb]
    wa, ha = _V(WHA[:, 0:F]), _V(WHA[:, F:F2])
    wb, hb = _V(WHB[:, 0:F]), _V(WHB[:, F:F2])

    I1 = tt2("I1", p1a, p1b, OP.max)                  # [ix1 | iy1]
    I2 = tt2("I2", p2a, p2b, OP.min)                  # [ix2 | iy2]
    IR = tt2("IR", I2, I1, OP.subtract)               # [iwr | ihr]
    s.drop("I1", "I2")
    iwr, ihr = _V(IR[:, 0:F]), _V(IR[:, F:F2])
    SW = tt2("SW", WHA, WHB, OP.add)                  # [swx | swy]
    CWH = tt2("CWH", SW, IR, OP.subtract)             # [cw | ch]
    s.drop("SW")
    cw, ch = _V(CWH[:, 0:F]), _V(CWH[:, F:F2])
    inter = s.alloc("inter")
    v._custom_dve(_BOX_OPS["RELUMUL"], out=inter[:], in0=iwr[:], in1=ihr[:])
    s.drop("IR")
    c2 = s.alloc("c2")
    v._custom_dve(_BOX_OPS["SQADD"], out=c2[:], in0=cw[:], in1=ch[:], s0=1.0)
    s.drop("CWH")

    D1 = tt2("D1", p1b, p1a, OP.subtract)             # [x1b-x1a | y1b-y1a]
    DW = tt2("DW", WHB, WHA, OP.subtract)             # [dwx | dwy]
    DR = s.alloc("DR", w=F2)
    v.affine_then_add(DR[:], D1[:], DW[:], 2.0, 0.0)  # [dxr | dyr]
    s.drop("D1", "DW")
    dxr, dyr = _V(DR[:, 0:F]), _V(DR[:, F:F2])
    rho2 = s.alloc("rho2")
    v._custom_dve(_BOX_OPS["SQADD"], out=rho2[:], in0=dxr[:], in1=dyr[:], s0=0.5)
    s.drop("DR")

    # --- union / iou ---
    area_a = tt("area_a", wa, ha, OP.mult)
    area_b = tt("area_b", wb, hb, OP.mult)
    ab = tt("ab", area_a, area_b, OP.add)
    s.drop("area_a", "area_b")
    union = tt("union", ab, inter, OP.subtract)
    s.drop("ab")
    r_union = s.alloc("r_union")
    v.reciprocal_approx_fast(r_union[:], union[:])
    s.drop("union")
    iou = tt("iou", inter, r_union, OP.mult)
    s.drop("inter", "r_union")

    # --- center distance term ---
    r_c2 = s.alloc("r_c2")
    v.reciprocal_approx_fast(r_c2[:], c2[:])
    s.drop("c2")
    term_rho = tt("term_rho", rho2, r_c2, OP.mult)
    s.drop("rho2", "r_c2")

    # --- aspect-ratio term ---
    n1 = tt("n1", wb, ha, OP.mult)
    n2 = tt("n2", wa, hb, OP.mult)
    num = tt("num", n1, n2, OP.subtract)
    s.drop("n1", "n2")
    dd1 = tt("dd1", ha, hb, OP.mult)
    dd2 = tt("dd2", wa, wb, OP.mult)
    den = tt("den", dd1, dd2, OP.add)
    s.drop("dd1", "dd2", "WHA", "WHB")
    r_den = s.alloc("r_den")
    v.reciprocal_approx_fast(r_den[:], den[:])
    s.drop("den")
    azc = s.alloc("azc")
    v._custom_dve(_BOX_OPS["ZABSC"], out=azc[:], in0=num[:], in1=r_den[:], s0=1e-12)
    s.drop("num", "r_den")
    rz = s.alloc("rz")
    v.reciprocal_approx_fast(rz[:], azc[:])
    a = tt("a", azc, rz, OP.min)
    s.drop("rz")
    t_at = s.alloc("t_at")
    sc.activation(t_at[:], a[:], AF.Arctan)
    s.drop("a")
    vv = s.alloc("vv")
    v._custom_dve(_BOX_OPS["VQ"], out=vv[:], in0=t_at[:], in1=azc[:],
                  s0=PI / 2.0, s1=2.0 / PI)
    s.drop("t_at", "azc")

    # --- alpha * v ---
    denom = s.alloc("denom")
    v._custom_dve(_BOX_OPS["DENOM"], out=denom[:], in0=vv[:], in1=iou[:], s0=EPS)
    r_denom = s.alloc("r_denom")
    v.reciprocal_approx_fast(r_denom[:], denom[:])
    s.drop("denom")
    alphav = s.alloc("alphav")
    v._custom_dve(_BOX_OPS["SQMUL"], out=alphav[:], in0=vv[:], in1=r_denom[:])
    s.drop("vv", "r_denom")

    # --- combine, mask, output, loss partial ---
    p1 = tt("p1", iou, term_rho, OP.subtract)
    s.drop("iou", "term_rho")
    p2 = tt("p2", p1, alphav, OP.subtract)
    s.drop("p1", "alphav")
    iou_out = tt("iou_out", p2, ins["m"], OP.mult)
    s.drop("p2")
    nc.sync.dma_start(iou_out_dram[:, ti * F:(ti + 1) * F], iou_out[:])

    scratch = s.alloc("scratch")
    v.scalar_tensor_tensor(
        scratch[:], iou_out[:], 0.0, ins["bn"][:], OP.add, OP.mult,
        accum_out=acc_out,
    )
    s.drop("scratch", "iou_out")


def build_bass(loop_n=None):
    """Build the per-core Bass program (identical on all 8 cores).

    loop_n: if set, wrap the whole program in a hardware For_i loop that
    re-runs it loop_n times (for timing measurement only — outputs are
    rewritten with identical values each iteration).
    """
    nc = bacc.Bacc("TRN2", target_bir_lowering=False)
    din = {
        nm: nc.dram_tensor(nm, [P, N], mybir.dt.float32, kind="ExternalInput").ap()
        for nm in IN_NAMES
    }
    iou_out_d = nc.dram_tensor(
        "iou_out", [P, N], mybir.dt.float32, kind="ExternalOutput"
    ).ap()
    acc_d = nc.dram_tensor("acc", [P, 1], mybir.dt.float32, kind="ExternalOutput").ap()
    if PE_OFF:
        wpos_d = nc.dram_tensor("wpos", [P, P], mybir.dt.float32,
                                kind="ExternalInput").ap()
        wneg_d = nc.dram_tensor("wneg", [P, P], mybir.dt.float32,
                                kind="ExternalInput").ap()

    with TileContext(nc) as tc:
        with (
            tc.tile_pool(name="inp", bufs=INP_BUFS) as inp_pool,
            tc.tile_pool(name="work", bufs=WORK_BUFS) as work_pool,
            tc.tile_pool(name="accp", bufs=1) as acc_pool,
            tc.tile_pool(name="ps", bufs=1, space="PSUM") as psum_pool,
            tc.tile_pool(name="wts", bufs=1) as wts_pool,
        ):
            pe = None
            if PE_OFF:
                wpos_t = wts_pool.tile([P, P], mybir.dt.float32, tag="wpos")
                wneg_t = wts_pool.tile([P, P], mybir.dt.float32, tag="wneg")
                nc.sync.dma_start(wpos_t[:], wpos_d[:])
                nc.sync.dma_start(wneg_t[:], wneg_d[:])
                pe = (psum_pool, wpos_t, wneg_t)
            def body(_iv=None):
                s = _Slots(work_pool, mybir.dt.float32)
                s_list = [_Slots(work_pool, mybir.dt.float32) for _ in range(T)]
                ins_list = [None] * T
                acc_tiles = []
                for ti in range(T):
                    sl = slice(ti * F, (ti + 1) * F)
                    ins = {}
                    if PACK_XY:
                        pairs = {
                            "p1a": ("x1a", "y1a"), "p2a": ("x2a", "y2a"),
                            "p1b": ("x1b", "y1b"), "p2b": ("x2b", "y2b"),
                        }
                        for pnm, (xn, yn) in pairs.items():
                            t = inp_pool.tile([P, 2 * F], mybir.dt.float32, tag=pnm)
                            nc.sync.dma_start(t[:, 0:F], din[xn][:, sl])
                            nc.sync.dma_start(t[:, F:2 * F], din[yn][:, sl])
                            ins[pnm] = t
                        for nm in ("m", "bn"):
                            t = inp_pool.tile([P, F], mybir.dt.float32, tag=nm)
                            nc.sync.dma_start(t[:], din[nm][:, sl])
                            ins[nm] = t
                    else:
                        half = F // 2
                        split_set = ("x1a", "x2a", "x1b", "x2b") \
                            if (SPLIT0 and ti == 0) else ()
                        for nm in split_set:  # first halves of first-needed
                            t = inp_pool.tile([P, F], mybir.dt.float32, tag=nm)
                            nc.sync.dma_start(
                                t[:, 0:half], din[nm][:, ti * F:ti * F + half])
                            ins[nm] = t
                        for nm in split_set:  # then their second halves
                            nc.sync.dma_start(
                                ins[nm][:, half:F],
                                din[nm][:, ti * F + half:(ti + 1) * F])
                        for nm in LOAD_ORDER:
                            if nm in split_set:
                                continue
                            t = inp_pool.tile([P, F], mybir.dt.float32, tag=nm)
                            nc.sync.dma_start(t[:], din[nm][:, sl])
                            ins[nm] = t
                    acc_t = acc_pool.tile([P, 1], mybir.dt.float32, tag=f"acc{ti}")
                    if PACK_XY:
                        _emit_tile_packed(nc, s, ins, iou_out_d, None, acc_t[:], ti)
                    elif ILV:
                        # phase 1 only (through arctan); post emitted below
                        ins_list[ti] = ins
                        _emit_tile(nc, s_list[ti], ins, iou_out_d, None,
                                   acc_t[:], ti, pe=pe, split=True)
                    else:
                        _emit_tile(nc, s, ins, iou_out_d, None, acc_t[:], ti, pe=pe)
                    acc_tiles.append(acc_t)
                if ILV and not PACK_XY:
                    for ti in range(T):
                        _emit_tile_post(nc, s_list[ti], ins_list[ti], iou_out_d,
                                        acc_tiles[ti][:], ti, pe)
                acc_total = acc_tiles[0]
                for ti in range(1, T):
                    acc_new = acc_pool.tile([P, 1], mybir.dt.float32, tag=f"accsum{ti}")
                    nc.vector.tensor_tensor(
                        acc_new[:], acc_total[:], acc_tiles[ti][:], OP.add
                    )
                    acc_total = acc_new
                nc.sync.dma_start(acc_d[:], acc_total[:])

            if loop_n is None:
                body()
            else:
                with tc.For_i(0, loop_n, 1):
                    body()
    nc.compile()
    return nc


_CACHE = {}


def _get_nc():
    if "nc" not in _CACHE:
        _CACHE["nc"] = build_bass()
    return _CACHE["nc"]


def shard_inputs(predicts_bbox, targets_bbox, valid_masks, box_norm):
    """Deinterleave xyxy into coordinate planes and shard over batch."""
    pb = np.asarray(predicts_bbox, dtype=np.float32)
    tb = np.asarray(targets_bbox, dtype=np.float32)
    m = np.asarray(valid_masks).astype(np.float32)
    bn = np.asarray(box_norm, dtype=np.float32)

    in_maps = []
    for c in range(NCORES):
        sl = slice(c * PER_B, (c + 1) * PER_B)
        d = {}
        for i, nm in enumerate(["x1a", "y1a", "x2a", "y2a"]):
            d[nm] = np.ascontiguousarray(pb[sl, :, i]).reshape(P, N)
        for i, nm in enumerate(["x1b", "y1b", "x2b", "y2b"]):
            d[nm] = np.ascontiguousarray(tb[sl, :, i]).reshape(P, N)
        d["m"] = np.ascontiguousarray(m[sl]).reshape(P, N)
        d["bn"] = np.ascontiguousarray(bn[sl]).reshape(P, N)
        if PE_OFF:
            d["wpos"] = np.eye(P, dtype=np.float32)
            d["wneg"] = -np.eye(P, dtype=np.float32)
        in_maps.append(d)
    return in_maps


def run_sharded(inputs, trace=False, trace_kwargs=None):
    """Run the SPMD kernel on 8 cores; returns (outputs, BassKernelResults)."""
    from concourse import bass_utils

    in_maps = shard_inputs(
        inputs["predicts_bbox"],
        inputs["targets_bbox"],
        inputs["valid_masks"],
        inputs["box_norm"],
    )
    nc = _get_nc()
    kw = {}
    if trace:
        kw["trace"] = True
        if trace_kwargs:
            kw.update(trace_kwargs)
    res = bass_utils.run_bass_kernel_spmd(
        nc, in_maps, core_ids=list(range(NCORES)), **kw
    )
    return res


def assemble_outputs(inputs, results):
    iou = np.concatenate(
        [r["iou_out"].reshape(PER_B, A) for r in results], axis=0
    ).astype(np.float32)

    m = np.asarray(inputs["valid_masks"]).astype(np.float64)
    bn = np.asarray(inputs["box_norm"], dtype=np.float64)
    s1 = float(np.sum(bn * m))                       # sum(box_norm * mask)
    s2 = float(sum(np.sum(r["acc"], dtype=np.float64) for r in results))
    cls_norm = float(np.asarray(inputs["cls_norm"]))
    loss = np.float32((s1 - s2) / cls_norm)
    return loss, iou


def kernel(**inputs):
    res = run_sharded(inputs)
    return assemble_outputs(inputs, res.results)
